# revision 18
# baseline (speedup 1.0000x reference)
"""ASTGCN block Trainium2 kernel — 8-core SPMD.

Sharding: core c handles batch b = c//2, node-row half h = c%2 (2000 rows).

Algebraic restructuring vs the reference:
  - Temporal attention E and the final t-mean fold into three 256x16
    matrices: spatial = Z0@(G0-G2) + Z1@G1 + (L@Z1)@(2*G2), where
    Z0 = P@x_flat, Z1 = L@Z0, and P = softmax_rows(Vs @ sigmoid-term).
  - sigmoid(s_lhs s_rhs^T + bs) (rank-16 + bias) is precomputed on host in
    fp32 and shipped as fp16; the 4*4000^3 S-matmul runs on-device in fp16
    with fp32 PSUM accumulation.
  - Softmax is two-pass (online row-max during the S-matmul, then
    exp/normalize); the normalization is deferred past the Z0 matmul.
  - Chebyshev cross-half dependencies use 2 pairwise AllGathers (2MB).
Host does only O(B*N*F*T) work: small attention tensors, time-conv,
residual, final LayerNorm+relu.
"""

import numpy as np

import concourse.bass as bass
import concourse.mybir as mybir
import concourse.tile as tile
from concourse import bacc
from concourse.bass_utils import run_bass_kernel_spmd
from concourse.masks import make_identity

B, N, FD, TD, OD = 4, 4000, 16, 16, 16
C_ = FD * TD          # 256 flattened (f,s) feature dim
R = N // 2            # 2000 rows per core
N_CORES = 8
LN_EPS = 1e-5

NP = 4096             # padded m-rows (zeros)
NT = 16               # n-tiles over R: 15*128 + 80
MB = 32               # m-blocks over N: 31*128 + 32
KC = 16               # k-chunks over N: 15*256 + 160
f32 = mybir.dt.float32
f16 = mybir.dt.float16
f32r = mybir.dt.float32r


def _ntw(i):
    return 128 if i < NT - 1 else R - 128 * (NT - 1)      # 80


def _kpw(i):
    return 128 if i < MB - 1 else N - 128 * (MB - 1)      # 32


def _kcw(i):
    return 256 if i < KC - 1 else N - 256 * (KC - 1)      # 160


def build_nc(single_core=False, reps=1, no_coll=False):
    nc = bacc.Bacc("TRN2", target_bir_lowering=False, debug=False,
                   num_devices=1 if single_core else N_CORES)

    sig_h = nc.dram_tensor("sig", [NP, N], f16, kind="ExternalInput")
    vst_h = nc.dram_tensor("vst", [NP, R], f16, kind="ExternalInput")
    x_h = nc.dram_tensor("xf", [NP, C_], f16, kind="ExternalInput")
    lt_h = nc.dram_tensor("lt", [NP, R], f16, kind="ExternalInput")
    gcat_h = nc.dram_tensor("gcat", [3 * C_, OD], f16, kind="ExternalInput")
    out_h = nc.dram_tensor("spatial", [R, OD], f32, kind="ExternalOutput")

    sig_r = sig_h.rearrange("(a p) k -> p a k", p=128)
    vst_r = vst_h.rearrange("(a p) j -> p a j", p=128)
    x_r = x_h.rearrange("(a p) c -> p a c", p=128)
    lt_r = lt_h.rearrange("(a p) j -> p a j", p=128)
    gcat_r = gcat_h.rearrange("(a p) o -> p a o", p=128)

    groups = [[0, 1], [2, 3], [4, 5], [6, 7]]
    AF = mybir.ActivationFunctionType

    with tile.TileContext(nc) as tc:
      for _rep in range(reps):
        with (
            tc.tile_pool(name="persist", bufs=1) as pp,
            tc.tile_pool(name="dram", bufs=1, space="DRAM") as dram,
        ):
            ident = pp.tile([128, 128], f16)
            make_identity(nc, ident[:])
            gcat_s = pp.tile([128, 6, OD], f16)
            nc.sync.dma_start(gcat_s[:], gcat_r[:])

            runmax = pp.tile([128, NT], f32)
            rowsum = pp.tile([128, NT], f32)
            recip = pp.tile([128, NT], f32)
            z0acc = pp.tile([128, NT, C_], f32)
            nc.vector.memset(runmax[:], -3.0e38)
            nc.vector.memset(rowsum[:], 0.0)
            nc.vector.memset(z0acc[:], 0.0)

            spat_acc = pp.tile([128, NT, OD], f32)
            nc.vector.memset(spat_acc[:], 0.0)

            z0_own = dram.tile([R, C_], f16)
            z0_full = dram.tile([N, C_], f16)
            z1_own = dram.tile([R, C_], f16)
            z1_full = dram.tile([N, C_], f16)
            lz1_dram = dram.tile([R, C_], f16)

            def spat_contrib(zsb, nt, nw, zi, et_pool, zt_pool, sp_ps_pool,
                             tags=("etp", "zt", "spo")):
                """spat_acc[:,nt,:] += zsb[128,C_] @ Gcat block zi (fp16)."""
                ops = sp_ps_pool.tile([128, OD], f32, tag=tags[2])
                for j in range(2):
                    tp = et_pool.tile([128, 128], f16, tag=tags[0])
                    nc.tensor.transpose(
                        tp[:128, :nw],
                        zsb[:nw, j * 128: (j + 1) * 128],
                        ident[:nw, :nw])
                    zt = zt_pool.tile([128, 128], f16, tag=tags[1])
                    nc.vector.tensor_copy(zt[:, :nw], tp[:, :nw])
                    nc.tensor.matmul(
                        ops[:nw, :], zt[:, :nw], gcat_s[:, zi * 2 + j, :],
                        start=(j == 0), stop=(j == 1))
                nc.vector.tensor_add(spat_acc[:nw, nt, :],
                                     spat_acc[:nw, nt, :], ops[:nw, :])

            # ---- Phase A: S-matmul + online softmax + Z0 accumulation ----
            with (
                tc.tile_pool(name="vst", bufs=1) as vst_pool,
                tc.tile_pool(name="sigp", bufs=2) as sig_pool,
                tc.tile_pool(name="xk", bufs=4) as xk_pool,
                tc.tile_pool(name="usb", bufs=3) as u_pool,
                tc.tile_pool(name="utsb", bufs=4) as ut_pool,
                tc.tile_pool(name="stat", bufs=4) as st_pool,
                tc.tile_pool(name="z0sb", bufs=2) as z0sb_pool,
                tc.tile_pool(name="zta", bufs=3) as zta_pool,
                tc.tile_pool(name="spsum", bufs=4, space="PSUM") as sp_pool,
                tc.tile_pool(name="tpsum", bufs=2, space="PSUM") as tp_pool,
                tc.tile_pool(name="cpsum", bufs=2, space="PSUM") as cp_pool,
            ):
                vst_chunks = []
                for g in range(4):
                    vc = vst_pool.tile([128, 8, R], f16, tag=f"vst{g}")
                    nc.sync.dma_start(vc[:], vst_r[:, g * 8:(g + 1) * 8, :])
                    vst_chunks.append(vc)

                for kc in range(KC):
                    kw = _kcw(kc)
                    nsub = (kw + 127) // 128
                    panel = sig_pool.tile([128, MB, 256], f16, tag="sig")
                    nc.sync.dma_start(
                        panel[:, :, :kw],
                        sig_r[:, :, kc * 256: kc * 256 + kw])
                    xks = []
                    for j in range(nsub):
                        kb = kc * 2 + j
                        jw = min(128, kw - j * 128)
                        xk = xk_pool.tile([128, C_], f16, tag="xk")
                        nc.sync.dma_start(xk[:jw, :], x_r[:jw, kb, :])
                        xks.append((xk, jw))

                    for nt in range(NT):
                        nw = _ntw(nt)
                        ps = sp_pool.tile([128, 256], f32, tag="sp")
                        for mb in range(MB):
                            nc.tensor.matmul(
                                ps[:nw, :kw],
                                vst_chunks[mb // 8][:, mb % 8,
                                                   nt * 128: nt * 128 + nw],
                                panel[:, mb, :kw],
                                start=(mb == 0), stop=(mb == MB - 1))
                        # online max / rescale
                        cmax = st_pool.tile([128, 1], f32, tag="cm")
                        nc.vector.reduce_max(cmax[:nw, :], ps[:nw, :kw],
                                             axis=mybir.AxisListType.X)
                        newmax = st_pool.tile([128, 1], f32, tag="nm")
                        nc.vector.tensor_max(newmax[:nw, :],
                                             runmax[:nw, nt: nt + 1],
                                             cmax[:nw, :])
                        diff = st_pool.tile([128, 1], f32, tag="df")
                        nc.vector.tensor_sub(diff[:nw, :],
                                             runmax[:nw, nt: nt + 1],
                                             newmax[:nw, :])
                        resc = st_pool.tile([128, 1], f32, tag="rs")
                        nc.scalar.activation(resc[:nw, :], diff[:nw, :],
                                             AF.Exp)
                        nc.vector.tensor_copy(runmax[:nw, nt: nt + 1],
                                              newmax[:nw, :])
                        negm = st_pool.tile([128, 1], f32, tag="ng")
                        nc.vector.tensor_scalar_mul(negm[:nw, :],
                                                    newmax[:nw, :], -1.0)
                        # exp + partial rowsum
                        ue = u_pool.tile([128, 256], f16, tag="u")
                        rs_part = st_pool.tile([128, 1], f32, tag="rp")
                        nc.scalar.activation(
                            ue[:nw, :kw], ps[:nw, :kw], AF.Exp,
                            bias=negm[:nw, :], accum_out=rs_part[:nw, :])
                        nc.vector.tensor_scalar_mul(rowsum[:nw, nt: nt + 1],
                                                    rowsum[:nw, nt: nt + 1],
                                                    resc[:nw, :])
                        nc.vector.tensor_add(rowsum[:nw, nt: nt + 1],
                                             rowsum[:nw, nt: nt + 1],
                                             rs_part[:nw, :])
                        # transpose exp'd tile, contrib = U^T @ X
                        contrib = cp_pool.tile([128, C_], f32, tag="cp")
                        for j in range(nsub):
                            jw = min(128, kw - j * 128)
                            tp = tp_pool.tile([128, 128], f16, tag="tp")
                            nc.tensor.transpose(
                                tp[:jw, :nw],
                                ue[:nw, j * 128: j * 128 + jw],
                                ident[:nw, :nw])
                            ut = ut_pool.tile([128, 128], f16, tag="ut")
                            nc.vector.tensor_copy(ut[:jw, :nw], tp[:jw, :nw])
                            nc.tensor.matmul(
                                contrib[:nw, :], ut[:jw, :nw],
                                xks[j][0][:jw, :],
                                start=(j == 0), stop=(j == nsub - 1))
                        # acc = acc * resc + contrib
                        nc.vector.tensor_scalar_mul(z0acc[:nw, nt, :],
                                                    z0acc[:nw, nt, :],
                                                    resc[:nw, :])
                        nc.vector.tensor_add(z0acc[:nw, nt, :],
                                             z0acc[:nw, nt, :],
                                             contrib[:nw, :])

                for nt in range(NT):
                    nw = _ntw(nt)
                    nc.vector.reciprocal(recip[:nw, nt: nt + 1],
                                         rowsum[:nw, nt: nt + 1])
                    z0sb = z0sb_pool.tile([128, C_], f16, tag="z0sb")
                    nc.vector.tensor_scalar_mul(z0sb[:nw, :],
                                                z0acc[:nw, nt, :],
                                                recip[:nw, nt: nt + 1])
                    nc.sync.dma_start(
                        z0_own[nt * 128: nt * 128 + nw, :], z0sb[:nw, :])
                    spat_contrib(z0sb, nt, nw, 0, tp_pool, zta_pool,
                                 cp_pool, tags=("tp", "zta", "cp"))

            if single_core or no_coll:
                nc.sync.dma_start(z0_full[:R, :], z0_own[:])
                nc.sync.dma_start(z0_full[R:, :], z0_own[:])
            else:
                nc.gpsimd.collective_compute(
                    "AllGather", mybir.AluOpType.bypass,
                    replica_groups=groups,
                    ins=[z0_own.opt()], outs=[z0_full.opt()])

            # ---- Phases C/D: Z1 = L@Z0_full, LZ1 = L@Z1_full -------------
            with (
                tc.tile_pool(name="ltres", bufs=1) as lt_pool,
                tc.tile_pool(name="etps2", bufs=2, space="PSUM") as et2_pool,
                tc.tile_pool(name="ztsb", bufs=3) as zt2_pool,
                tc.tile_pool(name="cpsum2", bufs=2, space="PSUM") as cp2_pool,
                tc.tile_pool(name="zf", bufs=1) as zf_pool,
                tc.tile_pool(name="zsb", bufs=2) as zsb_pool,
                tc.tile_pool(name="zpsum", bufs=2, space="PSUM") as zp_pool,
            ):
                lt_chunks = []
                for g in range(4):
                    lc = lt_pool.tile([128, 8, R], f16, tag=f"lt{g}")
                    nc.sync.dma_start(lc[:], lt_r[:, g * 8:(g + 1) * 8, :])
                    lt_chunks.append(lc)

                z0f_c = []
                for g in range(4):
                    zc = zf_pool.tile([128, 8, C_], f16, tag=f"zf{g}")
                    for m8 in range(8):
                        mb = g * 8 + m8
                        pw = _kpw(mb)
                        nc.sync.dma_start(
                            zc[:pw, m8, :],
                            z0_full[mb * 128: mb * 128 + pw, :])
                    z0f_c.append(zc)
                for nt in range(NT):
                    nw = _ntw(nt)
                    zps = zp_pool.tile([128, C_], f32, tag="z1")
                    for mb in range(MB):
                        pw = _kpw(mb)
                        nc.tensor.matmul(
                            zps[:nw, :],
                            lt_chunks[mb // 8][:pw, mb % 8,
                                               nt * 128: nt * 128 + nw],
                            z0f_c[mb // 8][:pw, mb % 8, :],
                            start=(mb == 0), stop=(mb == MB - 1))
                    z1sb = zsb_pool.tile([128, C_], f16, tag="zsb")
                    nc.vector.tensor_copy(z1sb[:nw, :], zps[:nw, :])
                    nc.sync.dma_start(
                        z1_own[nt * 128: nt * 128 + nw, :], z1sb[:nw, :])
                    spat_contrib(z1sb, nt, nw, 1, et2_pool, zt2_pool,
                                 cp2_pool)

                if single_core or no_coll:
                    nc.sync.dma_start(z1_full[:R, :], z1_own[:])
                    nc.sync.dma_start(z1_full[R:, :], z1_own[:])
                else:
                    nc.gpsimd.collective_compute(
                        "AllGather", mybir.AluOpType.bypass,
                        replica_groups=groups,
                        ins=[z1_own.opt()], outs=[z1_full.opt()])

                z1f_c = []
                for g in range(4):
                    zc = zf_pool.tile([128, 8, C_], f16, tag=f"zg{g}")
                    for m8 in range(8):
                        mb = g * 8 + m8
                        pw = _kpw(mb)
                        nc.sync.dma_start(
                            zc[:pw, m8, :],
                            z1_full[mb * 128: mb * 128 + pw, :])
                    z1f_c.append(zc)
                for nt in range(NT):
                    nw = _ntw(nt)
                    zps = zp_pool.tile([128, C_], f32, tag="z1")
                    for mb in range(MB):
                        pw = _kpw(mb)
                        nc.tensor.matmul(
                            zps[:nw, :],
                            lt_chunks[mb // 8][:pw, mb % 8,
                                               nt * 128: nt * 128 + nw],
                            z1f_c[mb // 8][:pw, mb % 8, :],
                            start=(mb == 0), stop=(mb == MB - 1))
                    lzsb = zsb_pool.tile([128, C_], f16, tag="zsb")
                    nc.vector.tensor_copy(lzsb[:nw, :], zps[:nw, :])
                    spat_contrib(lzsb, nt, nw, 2, et2_pool, zt2_pool,
                                 cp2_pool)
                    osb = zsb_pool.tile([128, OD], f32, tag="osb")
                    nc.vector.tensor_copy(osb[:nw, :], spat_acc[:nw, nt, :])
                    nc.sync.dma_start(
                        out_h[nt * 128: nt * 128 + nw, :], osb[:nw, :])

    nc.compile()
    return nc


_NC = None


def _get_nc():
    global _NC
    if _NC is None:
        _NC = build_nc()
    return _NC


def host_prep(x, laplacian, W1, W2, W3, bs, Vs, U1, U2, U3, be, Ve,
              cheb_w, time_w, time_b, ln_g, ln_b):
    x = np.asarray(x, np.float32)
    laplacian = np.asarray(laplacian, np.float32)

    # ---- host: temporal attention E -> folded Gcat matrices ----
    def _sigmoid(z):
        return 1.0 / (1.0 + np.exp(-z))

    t_lhs = np.tensordot(np.asarray(U1, np.float32), x, axes=([0], [1])) \
        .sum(axis=1)                                   # (B,T)
    u2 = np.asarray(U3, np.float32) @ np.asarray(U2, np.float32)   # (N,)
    t_rhs = np.tensordot(u2, x, axes=([0], [1])).sum(axis=1)       # (B,T)
    t_prod = t_lhs[:, :, None] * t_rhs[:, None, :]                 # (B,T,T)
    E_pre = np.einsum('ts,bsr->btr', np.asarray(Ve, np.float32),
                      _sigmoid(t_prod + np.asarray(be, np.float32)))
    E_pre = E_pre - E_pre.max(axis=-1, keepdims=True)
    E = np.exp(E_pre)
    E /= E.sum(axis=-1, keepdims=True)                             # (B,T,T)
    e_bar = E.mean(axis=1)                                         # (B,T)

    cw = np.asarray(cheb_w, np.float32)
    gcats = []
    for b in range(B):
        G = [(cw[k][:, None, :] * e_bar[b][None, :, None])
             .reshape(C_, OD).astype(np.float32) for k in range(3)]
        gcats.append(np.concatenate([G[0] - G[2], G[1], 2.0 * G[2]],
                                    axis=0))           # (768, OD)

    # ---- host: spatial-attention sigmoid term (fp16) ----
    xr = x.reshape(-1, TD)
    s_lhs = (xr @ np.asarray(W1, np.float32)).reshape(B, N, FD)
    xW3 = (xr @ np.asarray(W3, np.float32)).reshape(B, N, FD)
    s_rhs = xW3 @ np.asarray(W2, np.float32).T        # (B,N,F)
    bs0 = np.asarray(bs, np.float32)[0]
    sigs = []
    for b in range(B):
        sp = s_lhs[b] @ s_rhs[b].T
        sp += bs0
        sigs.append(_sigmoid(sp).astype(np.float16))

    # ---- host: time conv + residual ----
    time_out = (x.reshape(B * N, C_)
                @ np.asarray(time_w, np.float32).reshape(OD, C_).T
                ).reshape(B, N, OD) + np.asarray(time_b, np.float32)
    residual = x[:, :, :, TD - 1]                     # (B,N,O)

    # ---- device inputs ----
    VsT = np.ascontiguousarray(np.asarray(Vs, np.float32).T)
    LT = np.ascontiguousarray(laplacian.T)
    in_maps = []
    for c in range(N_CORES):
        b, h = c // 2, c % 2
        r0 = h * R
        sig_p = np.zeros((4096, N), np.float16)
        sig_p[:N] = sigs[b]
        vst_p = np.zeros((4096, R), np.float16)
        vst_p[:N] = VsT[:, r0:r0 + R]
        x_p = np.zeros((4096, C_), np.float16)
        x_p[:N] = x[b].reshape(N, C_)
        lt_p = np.zeros((4096, R), np.float16)
        lt_p[:N] = LT[:, r0:r0 + R]
        in_maps.append({
            "sig": sig_p,
            "vst": vst_p,
            "xf": x_p,
            "lt": lt_p,
            "gcat": gcats[b].astype(np.float16),
        })

    return in_maps, time_out, residual, np.asarray(ln_g, np.float32), \
        np.asarray(ln_b, np.float32)


def host_post(results, time_out, residual, ln_g, ln_b):
    spatial = np.empty((B, N, OD), np.float32)
    for c in range(N_CORES):
        b, h = c // 2, c % 2
        spatial[b, h * R:(h + 1) * R] = results[c]["spatial"]
    y = spatial + time_out + residual
    mean = y.mean(axis=(1, 2), keepdims=True)
    var = y.var(axis=(1, 2), keepdims=True)
    y = (y - mean) / np.sqrt(var + LN_EPS) * ln_g + ln_b
    return np.maximum(y, 0.0).astype(np.float32)


def kernel(**inputs):
    in_maps, time_out, residual, ln_g, ln_b = host_prep(**inputs)
    nc = _get_nc()
    res = run_bass_kernel_spmd(nc, in_maps, core_ids=list(range(N_CORES)))
    return host_post(res.results, time_out, residual, ln_g, ln_b)


# revision 25
# speedup vs baseline: 28.9300x; 28.9300x over previous
"""ASTGCN block Trainium2 kernel — 8-core SPMD.

Sharding: core c handles batch b = c//2, node-row half h = c%2 (2000 rows).

Algebraic restructuring vs the reference:
  - Temporal attention E and the final t-mean fold into three 256x16
    matrices: spatial = Z0@(G0-G2) + Z1@G1 + (L@Z1)@(2*G2), where
    Z0 = P@x_flat, Z1 = L@Z0, and P = softmax_rows(Vs @ sigmoid-term).
  - sigmoid(s_lhs s_rhs^T + bs) (rank-16 + bias) is precomputed on host in
    fp32 and shipped as fp16; the 4*4000^3 S-matmul runs on-device in fp16
    with fp32 PSUM accumulation.
  - Softmax is two-pass (online row-max during the S-matmul, then
    exp/normalize); the normalization is deferred past the Z0 matmul.
  - Chebyshev cross-half dependencies use 2 pairwise AllGathers (2MB).
Host does only O(B*N*F*T) work: small attention tensors, time-conv,
residual, final LayerNorm+relu.
"""

import numpy as np

import concourse.bass as bass
import concourse.mybir as mybir
import concourse.tile as tile
from concourse import bacc
from concourse.bass_utils import run_bass_kernel_spmd
from concourse.masks import make_identity

B, N, FD, TD, OD = 4, 4000, 16, 16, 16
C_ = FD * TD          # 256 flattened (f,s) feature dim
R = N // 2            # 2000 rows per core
N_CORES = 8
LN_EPS = 1e-5

NP = 4096             # padded m-rows (zeros)
NT = 16               # n-tiles over R: 15*128 + 80
MB = 32               # m-blocks over N: 31*128 + 32
KC = 16               # k-chunks over N: 15*256 + 160
f32 = mybir.dt.float32
f16 = mybir.dt.float16
f32r = mybir.dt.float32r


def _ntw(i):
    return 128 if i < NT - 1 else R - 128 * (NT - 1)      # 80


def _kpw(i):
    return 128 if i < MB - 1 else N - 128 * (MB - 1)      # 32


def _kcw(i):
    return 256 if i < KC - 1 else N - 256 * (KC - 1)      # 160


def build_nc(single_core=False, reps=1, no_coll=False):
    nc = bacc.Bacc("TRN2", target_bir_lowering=False, debug=False,
                   num_devices=1 if single_core else N_CORES)

    sig_h = nc.dram_tensor("sig", [NP, N], f16, kind="ExternalInput")
    vst_h = nc.dram_tensor("vst", [NP, R], f16, kind="ExternalInput")
    x_h = nc.dram_tensor("xf", [NP, C_], f16, kind="ExternalInput")
    lt_h = nc.dram_tensor("lt", [NP, R], f16, kind="ExternalInput")
    gcat_h = nc.dram_tensor("gcat", [3 * C_, OD], f16, kind="ExternalInput")
    out_h = nc.dram_tensor("spatial", [R, OD], f32, kind="ExternalOutput")

    sig_r = sig_h.rearrange("(a p) k -> p a k", p=128)
    vst_r = vst_h.rearrange("(a p) j -> p a j", p=128)
    x_r = x_h.rearrange("(a p) c -> p a c", p=128)
    lt_r = lt_h.rearrange("(a p) j -> p a j", p=128)
    gcat_r = gcat_h.rearrange("(a p) o -> p a o", p=128)

    groups = [[0, 1], [2, 3], [4, 5], [6, 7]]
    AF = mybir.ActivationFunctionType

    with tile.TileContext(nc) as tc:
      for _rep in range(reps):
        with (
            tc.tile_pool(name="persist", bufs=1) as pp,
            tc.tile_pool(name="dram", bufs=1, space="DRAM") as dram,
        ):
            ident = pp.tile([128, 128], f16)
            make_identity(nc, ident[:])
            gcat_s = pp.tile([128, 6, OD], f16)
            nc.sync.dma_start(gcat_s[:], gcat_r[:])

            runmax = pp.tile([128, NT], f32)
            rowsum = pp.tile([128, NT], f32)
            recip = pp.tile([128, NT], f32)
            z0acc = pp.tile([128, NT, C_], f32)
            nc.vector.memset(runmax[:], -3.0e38)
            nc.vector.memset(rowsum[:], 0.0)
            nc.vector.memset(z0acc[:], 0.0)

            spat_acc = pp.tile([128, NT, OD], f32)
            nc.vector.memset(spat_acc[:], 0.0)

            z0_own = dram.tile([R, C_], f16)
            z0_full = dram.tile([N, C_], f16)
            z1_own = dram.tile([R, C_], f16)
            z1_full = dram.tile([N, C_], f16)
            lz1_dram = dram.tile([R, C_], f16)

            def spat_contrib(zsb, nt, nw, zi, et_pool, zt_pool, sp_ps_pool,
                             tags=("etp", "zt", "spo")):
                """spat_acc[:,nt,:] += zsb[128,C_] @ Gcat block zi (fp16)."""
                ops = sp_ps_pool.tile([128, OD], f32, tag=tags[2])
                for j in range(2):
                    tp = et_pool.tile([128, 128], f16, tag=tags[0])
                    nc.tensor.transpose(
                        tp[:128, :nw],
                        zsb[:nw, j * 128: (j + 1) * 128],
                        ident[:nw, :nw])
                    zt = zt_pool.tile([128, 128], f16, tag=tags[1])
                    nc.vector.tensor_copy(zt[:, :nw], tp[:, :nw])
                    nc.tensor.matmul(
                        ops[:nw, :], zt[:, :nw], gcat_s[:, zi * 2 + j, :],
                        start=(j == 0), stop=(j == 1))
                nc.vector.tensor_add(spat_acc[:nw, nt, :],
                                     spat_acc[:nw, nt, :], ops[:nw, :])

            # ---- Phase A: S-matmul + online softmax + Z0 accumulation ----
            with (
                tc.tile_pool(name="vst", bufs=1) as vst_pool,
                tc.tile_pool(name="sigp", bufs=2) as sig_pool,
                tc.tile_pool(name="xk", bufs=4) as xk_pool,
                tc.tile_pool(name="usb", bufs=3) as u_pool,
                tc.tile_pool(name="utsb", bufs=4) as ut_pool,
                tc.tile_pool(name="stat", bufs=4) as st_pool,
                tc.tile_pool(name="z0sb", bufs=2) as z0sb_pool,
                tc.tile_pool(name="zta", bufs=3) as zta_pool,
                tc.tile_pool(name="spsum", bufs=4, space="PSUM") as sp_pool,
                tc.tile_pool(name="tpsum", bufs=2, space="PSUM") as tp_pool,
                tc.tile_pool(name="cpsum", bufs=2, space="PSUM") as cp_pool,
            ):
                vst_chunks = []
                for g in range(4):
                    vc = vst_pool.tile([128, 8, R], f16, tag=f"vst{g}")
                    nc.sync.dma_start(vc[:], vst_r[:, g * 8:(g + 1) * 8, :])
                    vst_chunks.append(vc)

                for kc in range(KC):
                    kw = _kcw(kc)
                    nsub = (kw + 127) // 128
                    panel = sig_pool.tile([128, MB, 256], f16, tag="sig")
                    nc.sync.dma_start(
                        panel[:, :, :kw],
                        sig_r[:, :, kc * 256: kc * 256 + kw])
                    xks = []
                    for j in range(nsub):
                        kb = kc * 2 + j
                        jw = min(128, kw - j * 128)
                        xk = xk_pool.tile([128, C_], f16, tag="xk")
                        nc.sync.dma_start(xk[:jw, :], x_r[:jw, kb, :])
                        xks.append((xk, jw))

                    for nt in range(NT):
                        nw = _ntw(nt)
                        ps = sp_pool.tile([128, 256], f32, tag="sp")
                        for mb in range(MB):
                            nc.tensor.matmul(
                                ps[:nw, :kw],
                                vst_chunks[mb // 8][:, mb % 8,
                                                   nt * 128: nt * 128 + nw],
                                panel[:, mb, :kw],
                                start=(mb == 0), stop=(mb == MB - 1))
                        # online max / rescale
                        cmax = st_pool.tile([128, 1], f32, tag="cm")
                        nc.vector.reduce_max(cmax[:nw, :], ps[:nw, :kw],
                                             axis=mybir.AxisListType.X)
                        newmax = st_pool.tile([128, 1], f32, tag="nm")
                        nc.vector.tensor_max(newmax[:nw, :],
                                             runmax[:nw, nt: nt + 1],
                                             cmax[:nw, :])
                        diff = st_pool.tile([128, 1], f32, tag="df")
                        nc.vector.tensor_sub(diff[:nw, :],
                                             runmax[:nw, nt: nt + 1],
                                             newmax[:nw, :])
                        resc = st_pool.tile([128, 1], f32, tag="rs")
                        nc.scalar.activation(resc[:nw, :], diff[:nw, :],
                                             AF.Exp)
                        nc.vector.tensor_copy(runmax[:nw, nt: nt + 1],
                                              newmax[:nw, :])
                        negm = st_pool.tile([128, 1], f32, tag="ng")
                        nc.vector.tensor_scalar_mul(negm[:nw, :],
                                                    newmax[:nw, :], -1.0)
                        # exp + partial rowsum
                        ue = u_pool.tile([128, 256], f16, tag="u")
                        rs_part = st_pool.tile([128, 1], f32, tag="rp")
                        nc.scalar.activation(
                            ue[:nw, :kw], ps[:nw, :kw], AF.Exp,
                            bias=negm[:nw, :], accum_out=rs_part[:nw, :])
                        nc.vector.tensor_scalar_mul(rowsum[:nw, nt: nt + 1],
                                                    rowsum[:nw, nt: nt + 1],
                                                    resc[:nw, :])
                        nc.vector.tensor_add(rowsum[:nw, nt: nt + 1],
                                             rowsum[:nw, nt: nt + 1],
                                             rs_part[:nw, :])
                        # transpose exp'd tile, contrib = U^T @ X
                        contrib = cp_pool.tile([128, C_], f32, tag="cp")
                        for j in range(nsub):
                            jw = min(128, kw - j * 128)
                            tp = tp_pool.tile([128, 128], f16, tag="tp")
                            nc.tensor.transpose(
                                tp[:jw, :nw],
                                ue[:nw, j * 128: j * 128 + jw],
                                ident[:nw, :nw])
                            ut = ut_pool.tile([128, 128], f16, tag="ut")
                            nc.vector.tensor_copy(ut[:jw, :nw], tp[:jw, :nw])
                            nc.tensor.matmul(
                                contrib[:nw, :], ut[:jw, :nw],
                                xks[j][0][:jw, :],
                                start=(j == 0), stop=(j == nsub - 1))
                        # acc = acc * resc + contrib
                        nc.vector.tensor_scalar_mul(z0acc[:nw, nt, :],
                                                    z0acc[:nw, nt, :],
                                                    resc[:nw, :])
                        nc.vector.tensor_add(z0acc[:nw, nt, :],
                                             z0acc[:nw, nt, :],
                                             contrib[:nw, :])
                        if kc == KC - 1:
                            nc.vector.reciprocal(recip[:nw, nt: nt + 1],
                                                 rowsum[:nw, nt: nt + 1])
                            z0sb = z0sb_pool.tile([128, C_], f16, tag="z0sb")
                            nc.vector.tensor_scalar_mul(
                                z0sb[:nw, :], z0acc[:nw, nt, :],
                                recip[:nw, nt: nt + 1])
                            nc.sync.dma_start(
                                z0_own[nt * 128: nt * 128 + nw, :],
                                z0sb[:nw, :])
                            spat_contrib(z0sb, nt, nw, 0, tp_pool, zta_pool,
                                         cp_pool, tags=("tp", "zta", "cp"))

            if single_core or no_coll:
                nc.sync.dma_start(z0_full[:R, :], z0_own[:])
                nc.sync.dma_start(z0_full[R:, :], z0_own[:])
            else:
                nc.gpsimd.collective_compute(
                    "AllGather", mybir.AluOpType.bypass,
                    replica_groups=groups,
                    ins=[z0_own.opt()], outs=[z0_full.opt()])

            # ---- Phases C/D: Z1 = L@Z0_full, LZ1 = L@Z1_full -------------
            with (
                tc.tile_pool(name="ltres", bufs=1) as lt_pool,
                tc.tile_pool(name="etps2", bufs=2, space="PSUM") as et2_pool,
                tc.tile_pool(name="ztsb", bufs=3) as zt2_pool,
                tc.tile_pool(name="cpsum2", bufs=2, space="PSUM") as cp2_pool,
                tc.tile_pool(name="zf", bufs=1) as zf_pool,
                tc.tile_pool(name="zsb", bufs=2) as zsb_pool,
                tc.tile_pool(name="zpsum", bufs=2, space="PSUM") as zp_pool,
            ):
                lt_chunks = []
                for g in range(4):
                    lc = lt_pool.tile([128, 8, R], f16, tag=f"lt{g}")
                    nc.sync.dma_start(lc[:], lt_r[:, g * 8:(g + 1) * 8, :])
                    lt_chunks.append(lc)

                z0f_c = []
                for g in range(4):
                    zc = zf_pool.tile([128, 8, C_], f16, tag=f"zf{g}")
                    for m8 in range(8):
                        mb = g * 8 + m8
                        pw = _kpw(mb)
                        nc.sync.dma_start(
                            zc[:pw, m8, :],
                            z0_full[mb * 128: mb * 128 + pw, :])
                    z0f_c.append(zc)
                for nt in range(NT):
                    nw = _ntw(nt)
                    zps = zp_pool.tile([128, C_], f32, tag="z1")
                    for mb in range(MB):
                        pw = _kpw(mb)
                        nc.tensor.matmul(
                            zps[:nw, :],
                            lt_chunks[mb // 8][:pw, mb % 8,
                                               nt * 128: nt * 128 + nw],
                            z0f_c[mb // 8][:pw, mb % 8, :],
                            start=(mb == 0), stop=(mb == MB - 1))
                    z1sb = zsb_pool.tile([128, C_], f16, tag="zsb")
                    nc.vector.tensor_copy(z1sb[:nw, :], zps[:nw, :])
                    nc.sync.dma_start(
                        z1_own[nt * 128: nt * 128 + nw, :], z1sb[:nw, :])
                    spat_contrib(z1sb, nt, nw, 1, et2_pool, zt2_pool,
                                 cp2_pool)

                if single_core or no_coll:
                    nc.sync.dma_start(z1_full[:R, :], z1_own[:])
                    nc.sync.dma_start(z1_full[R:, :], z1_own[:])
                else:
                    nc.gpsimd.collective_compute(
                        "AllGather", mybir.AluOpType.bypass,
                        replica_groups=groups,
                        ins=[z1_own.opt()], outs=[z1_full.opt()])

                z1f_c = []
                for g in range(4):
                    zc = zf_pool.tile([128, 8, C_], f16, tag=f"zg{g}")
                    for m8 in range(8):
                        mb = g * 8 + m8
                        pw = _kpw(mb)
                        nc.sync.dma_start(
                            zc[:pw, m8, :],
                            z1_full[mb * 128: mb * 128 + pw, :])
                    z1f_c.append(zc)
                for nt in range(NT):
                    nw = _ntw(nt)
                    zps = zp_pool.tile([128, C_], f32, tag="z1")
                    for mb in range(MB):
                        pw = _kpw(mb)
                        nc.tensor.matmul(
                            zps[:nw, :],
                            lt_chunks[mb // 8][:pw, mb % 8,
                                               nt * 128: nt * 128 + nw],
                            z1f_c[mb // 8][:pw, mb % 8, :],
                            start=(mb == 0), stop=(mb == MB - 1))
                    lzsb = zsb_pool.tile([128, C_], f16, tag="zsb")
                    nc.vector.tensor_copy(lzsb[:nw, :], zps[:nw, :])
                    spat_contrib(lzsb, nt, nw, 2, et2_pool, zt2_pool,
                                 cp2_pool)
                    osb = zsb_pool.tile([128, OD], f32, tag="osb")
                    nc.vector.tensor_copy(osb[:nw, :], spat_acc[:nw, nt, :])
                    nc.sync.dma_start(
                        out_h[nt * 128: nt * 128 + nw, :], osb[:nw, :])

    nc.compile()
    return nc


_NC = None


def _get_nc():
    global _NC
    if _NC is None:
        _NC = build_nc()
    return _NC


def host_prep(x, laplacian, W1, W2, W3, bs, Vs, U1, U2, U3, be, Ve,
              cheb_w, time_w, time_b, ln_g, ln_b):
    x = np.asarray(x, np.float32)
    laplacian = np.asarray(laplacian, np.float32)

    # ---- host: temporal attention E -> folded Gcat matrices ----
    def _sigmoid(z):
        return 1.0 / (1.0 + np.exp(-z))

    t_lhs = np.tensordot(np.asarray(U1, np.float32), x, axes=([0], [1])) \
        .sum(axis=1)                                   # (B,T)
    u2 = np.asarray(U3, np.float32) @ np.asarray(U2, np.float32)   # (N,)
    t_rhs = np.tensordot(u2, x, axes=([0], [1])).sum(axis=1)       # (B,T)
    t_prod = t_lhs[:, :, None] * t_rhs[:, None, :]                 # (B,T,T)
    E_pre = np.einsum('ts,bsr->btr', np.asarray(Ve, np.float32),
                      _sigmoid(t_prod + np.asarray(be, np.float32)))
    E_pre = E_pre - E_pre.max(axis=-1, keepdims=True)
    E = np.exp(E_pre)
    E /= E.sum(axis=-1, keepdims=True)                             # (B,T,T)
    e_bar = E.mean(axis=1)                                         # (B,T)

    cw = np.asarray(cheb_w, np.float32)
    gcats = []
    for b in range(B):
        G = [(cw[k][:, None, :] * e_bar[b][None, :, None])
             .reshape(C_, OD).astype(np.float32) for k in range(3)]
        gcats.append(np.concatenate([G[0] - G[2], G[1], 2.0 * G[2]],
                                    axis=0))           # (768, OD)

    # ---- host: spatial-attention sigmoid term (fp16) ----
    xr = x.reshape(-1, TD)
    s_lhs = (xr @ np.asarray(W1, np.float32)).reshape(B, N, FD)
    xW3 = (xr @ np.asarray(W3, np.float32)).reshape(B, N, FD)
    s_rhs = xW3 @ np.asarray(W2, np.float32).T        # (B,N,F)
    bs0 = np.asarray(bs, np.float32)[0]
    sigs = []
    try:
        import jax
        import jax.numpy as jnp
        _cpu = jax.devices("cpu")[0]

        @jax.jit
        def _sig16(sl, sr, bb):
            return jax.nn.sigmoid(sl @ sr.T + bb).astype(jnp.float16)

        with jax.default_device(_cpu):
            for b in range(B):
                sigs.append(np.asarray(_sig16(s_lhs[b], s_rhs[b], bs0)))
    except Exception:
        for b in range(B):
            sp = s_lhs[b] @ s_rhs[b].T
            sp += bs0
            sigs.append(_sigmoid(sp).astype(np.float16))

    # ---- host: time conv + residual ----
    time_out = (x.reshape(B * N, C_)
                @ np.asarray(time_w, np.float32).reshape(OD, C_).T
                ).reshape(B, N, OD) + np.asarray(time_b, np.float32)
    residual = x[:, :, :, TD - 1]                     # (B,N,O)

    # ---- device inputs ----
    VsT = np.ascontiguousarray(np.asarray(Vs, np.float32).T)
    LT = np.ascontiguousarray(laplacian.T)
    in_maps = []
    for c in range(N_CORES):
        b, h = c // 2, c % 2
        r0 = h * R
        sig_p = np.zeros((4096, N), np.float16)
        sig_p[:N] = sigs[b]
        vst_p = np.zeros((4096, R), np.float16)
        vst_p[:N] = VsT[:, r0:r0 + R]
        x_p = np.zeros((4096, C_), np.float16)
        x_p[:N] = x[b].reshape(N, C_)
        lt_p = np.zeros((4096, R), np.float16)
        lt_p[:N] = LT[:, r0:r0 + R]
        in_maps.append({
            "sig": sig_p,
            "vst": vst_p,
            "xf": x_p,
            "lt": lt_p,
            "gcat": gcats[b].astype(np.float16),
        })

    return in_maps, time_out, residual, np.asarray(ln_g, np.float32), \
        np.asarray(ln_b, np.float32)


def host_post(results, time_out, residual, ln_g, ln_b):
    spatial = np.empty((B, N, OD), np.float32)
    for c in range(N_CORES):
        b, h = c // 2, c % 2
        spatial[b, h * R:(h + 1) * R] = results[c]["spatial"]
    y = spatial + time_out + residual
    mean = y.mean(axis=(1, 2), keepdims=True)
    var = y.var(axis=(1, 2), keepdims=True)
    y = (y - mean) / np.sqrt(var + LN_EPS) * ln_g + ln_b
    return np.maximum(y, 0.0).astype(np.float32)


def kernel(**inputs):
    in_maps, time_out, residual, ln_g, ln_b = host_prep(**inputs)
    nc = _get_nc()
    res = run_bass_kernel_spmd(nc, in_maps, core_ids=list(range(N_CORES)))
    return host_post(res.results, time_out, residual, ln_g, ln_b)


# revision 27
# speedup vs baseline: 30.8945x; 1.0679x over previous
"""ASTGCN block Trainium2 kernel — 8-core SPMD.

Sharding: core c handles batch b = c//2, node-row half h = c%2 (2000 rows).

Algebraic restructuring vs the reference:
  - Temporal attention E and the final t-mean fold into three 256x16
    matrices: spatial = Z0@(G0-G2) + Z1@G1 + (L@Z1)@(2*G2), where
    Z0 = P@x_flat, Z1 = L@Z0, and P = softmax_rows(Vs @ sigmoid-term).
  - sigmoid(s_lhs s_rhs^T + bs) (rank-16 + bias) is precomputed on host in
    fp32 and shipped as fp16; the 4*4000^3 S-matmul runs on-device in fp16
    with fp32 PSUM accumulation.
  - Softmax is two-pass (online row-max during the S-matmul, then
    exp/normalize); the normalization is deferred past the Z0 matmul.
  - Chebyshev cross-half dependencies use 2 pairwise AllGathers (2MB).
Host does only O(B*N*F*T) work: small attention tensors, time-conv,
residual, final LayerNorm+relu.
"""

import numpy as np

import concourse.bass as bass
import concourse.mybir as mybir
import concourse.tile as tile
from concourse import bacc
from concourse.bass_utils import run_bass_kernel_spmd
from concourse.masks import make_identity

B, N, FD, TD, OD = 4, 4000, 16, 16, 16
C_ = FD * TD          # 256 flattened (f,s) feature dim
R = N // 2            # 2000 rows per core
N_CORES = 8
LN_EPS = 1e-5

NP = 4096             # padded m-rows (zeros)
NT = 16               # n-tiles over R: 15*128 + 80
MB = 32               # m-blocks over N: 31*128 + 32
KC = 11               # k-chunks over N: 10*384 + 160
KCW = 384
f32 = mybir.dt.float32
f16 = mybir.dt.float16
f32r = mybir.dt.float32r


def _ntw(i):
    return 128 if i < NT - 1 else R - 128 * (NT - 1)      # 80


def _kpw(i):
    return 128 if i < MB - 1 else N - 128 * (MB - 1)      # 32


def _kcw(i):
    return KCW if i < KC - 1 else N - KCW * (KC - 1)      # 160


def build_nc(single_core=False, reps=1, no_coll=False):
    nc = bacc.Bacc("TRN2", target_bir_lowering=False, debug=False,
                   num_devices=1 if single_core else N_CORES)

    sig_h = nc.dram_tensor("sig", [NP, N], f16, kind="ExternalInput")
    vst_h = nc.dram_tensor("vst", [NP, R], f16, kind="ExternalInput")
    x_h = nc.dram_tensor("xf", [NP, C_], f16, kind="ExternalInput")
    lt_h = nc.dram_tensor("lt", [NP, R], f16, kind="ExternalInput")
    gcat_h = nc.dram_tensor("gcat", [3 * C_, OD], f16, kind="ExternalInput")
    out_h = nc.dram_tensor("spatial", [R, OD], f32, kind="ExternalOutput")

    sig_r = sig_h.rearrange("(a p) k -> p a k", p=128)
    vst_r = vst_h.rearrange("(a p) j -> p a j", p=128)
    x_r = x_h.rearrange("(a p) c -> p a c", p=128)
    lt_r = lt_h.rearrange("(a p) j -> p a j", p=128)
    gcat_r = gcat_h.rearrange("(a p) o -> p a o", p=128)

    groups = [[0, 1], [2, 3], [4, 5], [6, 7]]
    AF = mybir.ActivationFunctionType

    with tile.TileContext(nc) as tc:
      for _rep in range(reps):
        with (
            tc.tile_pool(name="persist", bufs=1) as pp,
            tc.tile_pool(name="dram", bufs=1, space="DRAM") as dram,
        ):
            ident = pp.tile([128, 128], f16)
            make_identity(nc, ident[:])
            gcat_s = pp.tile([128, 6, OD], f16)
            nc.sync.dma_start(gcat_s[:], gcat_r[:])

            runmax = pp.tile([128, NT], f32)
            rowsum = pp.tile([128, NT], f32)
            recip = pp.tile([128, NT], f32)
            z0acc = pp.tile([128, NT, C_], f16)
            nc.vector.memset(runmax[:], -3.0e38)
            nc.vector.memset(rowsum[:], 0.0)
            nc.vector.memset(z0acc[:], 0.0)

            spat_acc = pp.tile([128, NT, OD], f32)
            nc.vector.memset(spat_acc[:], 0.0)

            z0_own = dram.tile([R, C_], f16)
            z0_full = dram.tile([N, C_], f16)
            z1_own = dram.tile([R, C_], f16)
            z1_full = dram.tile([N, C_], f16)
            lz1_dram = dram.tile([R, C_], f16)

            def spat_contrib(zsb, nt, nw, zi, et_pool, zt_pool, sp_ps_pool,
                             tags=("etp", "zt", "spo")):
                """spat_acc[:,nt,:] += zsb[128,C_] @ Gcat block zi (fp16)."""
                ops = sp_ps_pool.tile([128, OD], f32, tag=tags[2])
                for j in range(2):
                    tp = et_pool.tile([128, 128], f16, tag=tags[0])
                    nc.tensor.transpose(
                        tp[:128, :nw],
                        zsb[:nw, j * 128: (j + 1) * 128],
                        ident[:nw, :nw])
                    zt = zt_pool.tile([128, 128], f16, tag=tags[1])
                    nc.vector.tensor_copy(zt[:, :nw], tp[:, :nw])
                    nc.tensor.matmul(
                        ops[:nw, :], zt[:, :nw], gcat_s[:, zi * 2 + j, :],
                        start=(j == 0), stop=(j == 1))
                nc.vector.tensor_add(spat_acc[:nw, nt, :],
                                     spat_acc[:nw, nt, :], ops[:nw, :])

            # ---- Phase A: S-matmul + online softmax + Z0 accumulation ----
            with (
                tc.tile_pool(name="vst", bufs=1) as vst_pool,
                tc.tile_pool(name="sigp", bufs=2) as sig_pool,
                tc.tile_pool(name="xk", bufs=4) as xk_pool,
                tc.tile_pool(name="usb", bufs=3) as u_pool,
                tc.tile_pool(name="utsb", bufs=4) as ut_pool,
                tc.tile_pool(name="stat", bufs=4) as st_pool,
                tc.tile_pool(name="z0sb", bufs=2) as z0sb_pool,
                tc.tile_pool(name="zta", bufs=3) as zta_pool,
                tc.tile_pool(name="spsum", bufs=4, space="PSUM") as sp_pool,
                tc.tile_pool(name="tpsum", bufs=2, space="PSUM") as tp_pool,
                tc.tile_pool(name="cpsum", bufs=2, space="PSUM") as cp_pool,
            ):
                vst_chunks = []
                for g in range(4):
                    vc = vst_pool.tile([128, 8, R], f16, tag=f"vst{g}")
                    nc.sync.dma_start(vc[:], vst_r[:, g * 8:(g + 1) * 8, :])
                    vst_chunks.append(vc)

                for kc in range(KC):
                    kw = _kcw(kc)
                    nsub = (kw + 127) // 128
                    panel = sig_pool.tile([128, MB, KCW], f16, tag="sig")
                    nc.sync.dma_start(
                        panel[:, :, :kw],
                        sig_r[:, :, kc * KCW: kc * KCW + kw])
                    xks = []
                    for j in range(nsub):
                        kb = kc * 3 + j
                        jw = min(128, kw - j * 128)
                        xk = xk_pool.tile([128, C_], f16, tag="xk")
                        nc.sync.dma_start(xk[:jw, :], x_r[:jw, kb, :])
                        xks.append((xk, jw))

                    for nt in range(NT):
                        nw = _ntw(nt)
                        ps = sp_pool.tile([128, KCW], f32, tag="sp")
                        for mb in range(MB):
                            nc.tensor.matmul(
                                ps[:nw, :kw],
                                vst_chunks[mb // 8][:, mb % 8,
                                                   nt * 128: nt * 128 + nw],
                                panel[:, mb, :kw],
                                start=(mb == 0), stop=(mb == MB - 1))
                        # online max / rescale
                        cmax = st_pool.tile([128, 1], f32, tag="cm")
                        nc.vector.reduce_max(cmax[:nw, :], ps[:nw, :kw],
                                             axis=mybir.AxisListType.X)
                        newmax = st_pool.tile([128, 1], f32, tag="nm")
                        nc.vector.tensor_max(newmax[:nw, :],
                                             runmax[:nw, nt: nt + 1],
                                             cmax[:nw, :])
                        diff = st_pool.tile([128, 1], f32, tag="df")
                        nc.vector.tensor_sub(diff[:nw, :],
                                             runmax[:nw, nt: nt + 1],
                                             newmax[:nw, :])
                        resc = st_pool.tile([128, 1], f32, tag="rs")
                        nc.scalar.activation(resc[:nw, :], diff[:nw, :],
                                             AF.Exp)
                        nc.vector.tensor_copy(runmax[:nw, nt: nt + 1],
                                              newmax[:nw, :])
                        negm = st_pool.tile([128, 1], f32, tag="ng")
                        nc.vector.tensor_scalar_mul(negm[:nw, :],
                                                    newmax[:nw, :], -1.0)
                        # exp + partial rowsum
                        ue = u_pool.tile([128, KCW], f16, tag="u")
                        rs_part = st_pool.tile([128, 1], f32, tag="rp")
                        nc.scalar.activation(
                            ue[:nw, :kw], ps[:nw, :kw], AF.Exp,
                            bias=negm[:nw, :], accum_out=rs_part[:nw, :])
                        nc.vector.tensor_scalar_mul(rowsum[:nw, nt: nt + 1],
                                                    rowsum[:nw, nt: nt + 1],
                                                    resc[:nw, :])
                        nc.vector.tensor_add(rowsum[:nw, nt: nt + 1],
                                             rowsum[:nw, nt: nt + 1],
                                             rs_part[:nw, :])
                        # transpose exp'd tile, contrib = U^T @ X
                        contrib = cp_pool.tile([128, C_], f32, tag="cp")
                        for j in range(nsub):
                            jw = min(128, kw - j * 128)
                            tp = tp_pool.tile([128, 128], f16, tag="tp")
                            nc.tensor.transpose(
                                tp[:jw, :nw],
                                ue[:nw, j * 128: j * 128 + jw],
                                ident[:nw, :nw])
                            ut = ut_pool.tile([128, 128], f16, tag="ut")
                            nc.vector.tensor_copy(ut[:jw, :nw], tp[:jw, :nw])
                            nc.tensor.matmul(
                                contrib[:nw, :], ut[:jw, :nw],
                                xks[j][0][:jw, :],
                                start=(j == 0), stop=(j == nsub - 1))
                        # acc = acc * resc + contrib
                        nc.vector.tensor_scalar_mul(z0acc[:nw, nt, :],
                                                    z0acc[:nw, nt, :],
                                                    resc[:nw, :])
                        nc.vector.tensor_add(z0acc[:nw, nt, :],
                                             z0acc[:nw, nt, :],
                                             contrib[:nw, :])
                        if kc == KC - 1:
                            nc.vector.reciprocal(recip[:nw, nt: nt + 1],
                                                 rowsum[:nw, nt: nt + 1])
                            z0sb = z0sb_pool.tile([128, C_], f16, tag="z0sb")
                            nc.vector.tensor_scalar_mul(
                                z0sb[:nw, :], z0acc[:nw, nt, :],
                                recip[:nw, nt: nt + 1])
                            nc.sync.dma_start(
                                z0_own[nt * 128: nt * 128 + nw, :],
                                z0sb[:nw, :])
                            spat_contrib(z0sb, nt, nw, 0, tp_pool, zta_pool,
                                         cp_pool, tags=("tp", "zta", "cp"))

            if single_core or no_coll:
                nc.sync.dma_start(z0_full[:R, :], z0_own[:])
                nc.sync.dma_start(z0_full[R:, :], z0_own[:])
            else:
                nc.gpsimd.collective_compute(
                    "AllGather", mybir.AluOpType.bypass,
                    replica_groups=groups,
                    ins=[z0_own.opt()], outs=[z0_full.opt()])

            # ---- Phases C/D: Z1 = L@Z0_full, LZ1 = L@Z1_full -------------
            with (
                tc.tile_pool(name="ltres", bufs=1) as lt_pool,
                tc.tile_pool(name="etps2", bufs=2, space="PSUM") as et2_pool,
                tc.tile_pool(name="ztsb", bufs=3) as zt2_pool,
                tc.tile_pool(name="cpsum2", bufs=2, space="PSUM") as cp2_pool,
                tc.tile_pool(name="zf", bufs=1) as zf_pool,
                tc.tile_pool(name="zsb", bufs=2) as zsb_pool,
                tc.tile_pool(name="zpsum", bufs=2, space="PSUM") as zp_pool,
            ):
                lt_chunks = []
                for g in range(4):
                    lc = lt_pool.tile([128, 8, R], f16, tag=f"lt{g}")
                    nc.sync.dma_start(lc[:], lt_r[:, g * 8:(g + 1) * 8, :])
                    lt_chunks.append(lc)

                z0f_c = []
                for g in range(4):
                    zc = zf_pool.tile([128, 8, C_], f16, tag=f"zf{g}")
                    for m8 in range(8):
                        mb = g * 8 + m8
                        pw = _kpw(mb)
                        nc.sync.dma_start(
                            zc[:pw, m8, :],
                            z0_full[mb * 128: mb * 128 + pw, :])
                    z0f_c.append(zc)
                for nt in range(NT):
                    nw = _ntw(nt)
                    zps = zp_pool.tile([128, C_], f32, tag="z1")
                    for mb in range(MB):
                        pw = _kpw(mb)
                        nc.tensor.matmul(
                            zps[:nw, :],
                            lt_chunks[mb // 8][:pw, mb % 8,
                                               nt * 128: nt * 128 + nw],
                            z0f_c[mb // 8][:pw, mb % 8, :],
                            start=(mb == 0), stop=(mb == MB - 1))
                    z1sb = zsb_pool.tile([128, C_], f16, tag="zsb")
                    nc.vector.tensor_copy(z1sb[:nw, :], zps[:nw, :])
                    nc.sync.dma_start(
                        z1_own[nt * 128: nt * 128 + nw, :], z1sb[:nw, :])
                    spat_contrib(z1sb, nt, nw, 1, et2_pool, zt2_pool,
                                 cp2_pool)

                if single_core or no_coll:
                    nc.sync.dma_start(z1_full[:R, :], z1_own[:])
                    nc.sync.dma_start(z1_full[R:, :], z1_own[:])
                else:
                    nc.gpsimd.collective_compute(
                        "AllGather", mybir.AluOpType.bypass,
                        replica_groups=groups,
                        ins=[z1_own.opt()], outs=[z1_full.opt()])

                z1f_c = []
                for g in range(4):
                    zc = zf_pool.tile([128, 8, C_], f16, tag=f"zg{g}")
                    for m8 in range(8):
                        mb = g * 8 + m8
                        pw = _kpw(mb)
                        nc.sync.dma_start(
                            zc[:pw, m8, :],
                            z1_full[mb * 128: mb * 128 + pw, :])
                    z1f_c.append(zc)
                for nt in range(NT):
                    nw = _ntw(nt)
                    zps = zp_pool.tile([128, C_], f32, tag="z1")
                    for mb in range(MB):
                        pw = _kpw(mb)
                        nc.tensor.matmul(
                            zps[:nw, :],
                            lt_chunks[mb // 8][:pw, mb % 8,
                                               nt * 128: nt * 128 + nw],
                            z1f_c[mb // 8][:pw, mb % 8, :],
                            start=(mb == 0), stop=(mb == MB - 1))
                    lzsb = zsb_pool.tile([128, C_], f16, tag="zsb")
                    nc.vector.tensor_copy(lzsb[:nw, :], zps[:nw, :])
                    spat_contrib(lzsb, nt, nw, 2, et2_pool, zt2_pool,
                                 cp2_pool)
                    osb = zsb_pool.tile([128, OD], f32, tag="osb")
                    nc.vector.tensor_copy(osb[:nw, :], spat_acc[:nw, nt, :])
                    nc.sync.dma_start(
                        out_h[nt * 128: nt * 128 + nw, :], osb[:nw, :])

    nc.compile()
    return nc


_NC = None


def _get_nc():
    global _NC
    if _NC is None:
        _NC = build_nc()
    return _NC


def host_prep(x, laplacian, W1, W2, W3, bs, Vs, U1, U2, U3, be, Ve,
              cheb_w, time_w, time_b, ln_g, ln_b):
    x = np.asarray(x, np.float32)
    laplacian = np.asarray(laplacian, np.float32)

    # ---- host: temporal attention E -> folded Gcat matrices ----
    def _sigmoid(z):
        return 1.0 / (1.0 + np.exp(-z))

    t_lhs = np.tensordot(np.asarray(U1, np.float32), x, axes=([0], [1])) \
        .sum(axis=1)                                   # (B,T)
    u2 = np.asarray(U3, np.float32) @ np.asarray(U2, np.float32)   # (N,)
    t_rhs = np.tensordot(u2, x, axes=([0], [1])).sum(axis=1)       # (B,T)
    t_prod = t_lhs[:, :, None] * t_rhs[:, None, :]                 # (B,T,T)
    E_pre = np.einsum('ts,bsr->btr', np.asarray(Ve, np.float32),
                      _sigmoid(t_prod + np.asarray(be, np.float32)))
    E_pre = E_pre - E_pre.max(axis=-1, keepdims=True)
    E = np.exp(E_pre)
    E /= E.sum(axis=-1, keepdims=True)                             # (B,T,T)
    e_bar = E.mean(axis=1)                                         # (B,T)

    cw = np.asarray(cheb_w, np.float32)
    gcats = []
    for b in range(B):
        G = [(cw[k][:, None, :] * e_bar[b][None, :, None])
             .reshape(C_, OD).astype(np.float32) for k in range(3)]
        gcats.append(np.concatenate([G[0] - G[2], G[1], 2.0 * G[2]],
                                    axis=0))           # (768, OD)

    # ---- host: spatial-attention sigmoid term (fp16) ----
    xr = x.reshape(-1, TD)
    s_lhs = (xr @ np.asarray(W1, np.float32)).reshape(B, N, FD)
    xW3 = (xr @ np.asarray(W3, np.float32)).reshape(B, N, FD)
    s_rhs = xW3 @ np.asarray(W2, np.float32).T        # (B,N,F)
    bs0 = np.asarray(bs, np.float32)[0]
    sigs = []
    try:
        import jax
        import jax.numpy as jnp
        _cpu = jax.devices("cpu")[0]

        @jax.jit
        def _sig16(sl, sr, bb):
            return jax.nn.sigmoid(sl @ sr.T + bb).astype(jnp.float16)

        with jax.default_device(_cpu):
            for b in range(B):
                sigs.append(np.asarray(_sig16(s_lhs[b], s_rhs[b], bs0)))
    except Exception:
        for b in range(B):
            sp = s_lhs[b] @ s_rhs[b].T
            sp += bs0
            sigs.append(_sigmoid(sp).astype(np.float16))

    # ---- host: time conv + residual ----
    time_out = (x.reshape(B * N, C_)
                @ np.asarray(time_w, np.float32).reshape(OD, C_).T
                ).reshape(B, N, OD) + np.asarray(time_b, np.float32)
    residual = x[:, :, :, TD - 1]                     # (B,N,O)

    # ---- device inputs ----
    VsT = np.ascontiguousarray(np.asarray(Vs, np.float32).T)
    LT = np.ascontiguousarray(laplacian.T)
    in_maps = []
    for c in range(N_CORES):
        b, h = c // 2, c % 2
        r0 = h * R
        sig_p = np.zeros((4096, N), np.float16)
        sig_p[:N] = sigs[b]
        vst_p = np.zeros((4096, R), np.float16)
        vst_p[:N] = VsT[:, r0:r0 + R]
        x_p = np.zeros((4096, C_), np.float16)
        x_p[:N] = x[b].reshape(N, C_)
        lt_p = np.zeros((4096, R), np.float16)
        lt_p[:N] = LT[:, r0:r0 + R]
        in_maps.append({
            "sig": sig_p,
            "vst": vst_p,
            "xf": x_p,
            "lt": lt_p,
            "gcat": gcats[b].astype(np.float16),
        })

    return in_maps, time_out, residual, np.asarray(ln_g, np.float32), \
        np.asarray(ln_b, np.float32)


def host_post(results, time_out, residual, ln_g, ln_b):
    spatial = np.empty((B, N, OD), np.float32)
    for c in range(N_CORES):
        b, h = c // 2, c % 2
        spatial[b, h * R:(h + 1) * R] = results[c]["spatial"]
    y = spatial + time_out + residual
    mean = y.mean(axis=(1, 2), keepdims=True)
    var = y.var(axis=(1, 2), keepdims=True)
    y = (y - mean) / np.sqrt(var + LN_EPS) * ln_g + ln_b
    return np.maximum(y, 0.0).astype(np.float32)


def kernel(**inputs):
    in_maps, time_out, residual, ln_g, ln_b = host_prep(**inputs)
    nc = _get_nc()
    res = run_bass_kernel_spmd(nc, in_maps, core_ids=list(range(N_CORES)))
    return host_post(res.results, time_out, residual, ln_g, ln_b)
